# revision 1
# baseline (speedup 1.0000x reference)
"""Trainium2 Bass kernel for nn_BinaryTokenClassificationModel (segment_reduce).

Math: logits[b,i,j] = dot(segmean(1+i), w_src) + dot(segmean(513+j), w_tgt) + bias,
where segmean(s) is the mean of outputs[b] over the s-th consecutive run of equal
word_ids (attention_mask is all ones for this problem).  dot commutes with the
segment mean, so per-token projections proj[t,c]=x[t]·w_c suffice: DVE multiplies
each x tile by the replicated weight row, the scalar engine's fused
activation-accumulate reduces it to per-token dots, gpsimd builds the pooling rhs
(s_hi one-hot x proj), and PE accumulates the ragged segment-sums with a factored
one-hot matmul (s_lo=seg%128 one-hot stationary).  The [512,512] broadcast-add
output is assembled with tiny selector matmuls.  Tokens whose segment id exceeds
1024 can never influence the output, so only the first NT*128 tokens
(host-computed cutoff) are ever loaded — the DMA roofline drops accordingly.
Per-token segment labels (s_lo and the s_hi one-hot staircase) are tiny
word_ids-derived index metadata and are staged from the host alongside the
shard/cutoff/crossover structure.

Sharding: pure data parallel, one example (B=8) per NeuronCore (8 cores).
"""
import sys

for _p in ("/opt/trn_rl_repo", "/root/.axon_site/_ro/trn_rl_repo"):
    if _p not in sys.path:
        sys.path.append(_p)

from contextlib import ExitStack

import numpy as np

import concourse.bacc as bacc
import concourse.bass as bass
import concourse.tile as tile
from concourse import mybir
from concourse.bass_utils import run_bass_kernel_spmd

F32 = mybir.dt.float32
BF16 = mybir.dt.bfloat16
P = 128
H = 1024
HC = H // P          # 8 h-chunks
NSH = 9              # s_hi one-hot width (covers segments 0..1151 >= 1..1024 needed)
NR = 3 * NSH         # pooling rhs width: (src, tgt, count) x 9
AL = mybir.AluOpType


def _build_nc(NT: int, modes: list[str]) -> bass.Bass:
    nc = bacc.Bacc("TRN2", target_bir_lowering=False, debug=False, num_devices=8)
    NCC = 4 * P + 10 * NT + 1
    x_d = nc.declare_dram_parameter("x", [NT * P, H], F32, isOutput=False)
    cc_d = nc.declare_dram_parameter("consts", [P, NCC], F32, isOutput=False)
    wb_d = nc.declare_dram_parameter("wrepb", [P, 2 * H], F32, isOutput=False)
    y_d = nc.declare_dram_parameter("y", [512, 512], F32, isOutput=True)

    with tile.TileContext(nc) as tc, ExitStack() as ctx:
        consts = ctx.enter_context(tc.tile_pool(name="consts", bufs=1))
        segp = ctx.enter_context(tc.tile_pool(name="segp", bufs=1))
        xpool = ctx.enter_context(tc.tile_pool(name="xp", bufs=7))
        scrp = ctx.enter_context(tc.tile_pool(name="scr", bufs=6))
        rpool = ctx.enter_context(tc.tile_pool(name="rp", bufs=3))
        vpool = ctx.enter_context(tc.tile_pool(name="vp", bufs=4))
        opool = ctx.enter_context(tc.tile_pool(name="op", bufs=4))
        ppool_acc = ctx.enter_context(tc.tile_pool(name="pacc", bufs=1, space="PSUM"))
        ppool_sm = ctx.enter_context(tc.tile_pool(name="psm", bufs=4, space="PSUM"))

        # ---- x stream owns the sync queue from t=0; w_src half (gates the
        # first multiply) leads the scalar queue, then consts, then w_tgt ----
        wrep = consts.tile([P, 2 * H], F32)        # [128, 2048]: w_src | w_tgt replicated rows
        nc.scalar.dma_start(out=wrep[:, 0:H], in_=wb_d[:, 0:H])
        cc = consts.tile([P, NCC], F32)
        nc.scalar.dma_start(out=cc, in_=cc_d[:])
        nc.scalar.dma_start(out=wrep[:, H:2 * H], in_=wb_d[:, H:2 * H])
        ident = cc[:, 0:P]
        s1 = cc[:, P:2 * P]
        s2 = cc[:, 2 * P:3 * P]
        iota = cc[:, 3 * P:4 * P]
        slo = cc[:, 4 * P:4 * P + NT]              # host-computed seg%128 per token
        ch_all = cc[:, 4 * P + NT:4 * P + 10 * NT].rearrange("p (i u) -> p i u", u=NSH)
        biascol = cc[:, NCC - 1:NCC]               # bias replicated down all partitions

        # s_lo one-hots for every tile in ONE fused compare; emitted inside the
        # main loop after the first pair's multiplies (fills a DMA-wait gap)
        cl_all = segp.tile([P, NT, P], F32)
        cls = [cl_all[:, i, :] for i in range(NT)]

        def emit_cl_all():
            nc.vector.tensor_tensor(
                out=cl_all,
                in0=iota.unsqueeze(1).to_broadcast((P, NT, P)),
                in1=slo.unsqueeze(2).to_broadcast((P, NT, P)),
                op=AL.is_equal)

        # ---- main loop over token tiles ----
        # proj[t, c] = x[t] . w_c via DVE multiply + ACT fused reduce; the
        # src/tgt crossover (host-computed per tile) avoids computing both
        # dots for most tiles.
        pool_ps = ppool_acc.tile([P, NR], F32)
        deferred = []
        # main loop: DVE multiplies only; ACT reduces; gpsimd builds the
        # pooling rhs; PE accumulates pool^T[(u,c), s_lo]
        for g in range(NT // 2):
            x_pair = xpool.tile([P, 2, H], F32)
            src = x_d[256 * g:256 * (g + 1), :].rearrange("(two p) h -> p two h", p=P)
            nc.sync.dma_start(out=x_pair, in_=src)
            for half in range(2):
                i = 2 * g + half
                x_sub = x_pair[:, half, :]
                v = vpool.tile([P, 2], F32)
                nc.gpsimd.memset(v, 0.0)
                for c in range(2):
                    if (c == 0 and modes[i] == "tgt") or (c == 1 and modes[i] == "src"):
                        continue
                    scr = scrp.tile([P, H], F32)
                    nc.vector.tensor_tensor(out=scr, in0=x_sub, in1=wrep[:, c * H:(c + 1) * H], op=AL.mult)
                    nc.scalar.activation(out=scr, in_=scr, func=mybir.ActivationFunctionType.Copy,
                                         accum_out=v[:, c:c + 1])
                ch = ch_all[:, i, :]
                r_t = rpool.tile([P, NSH, 3], F32, tag="r")
                nc.gpsimd.tensor_tensor(
                    out=r_t[:, :, 0:2],
                    in0=ch.unsqueeze(2).to_broadcast((P, NSH, 2)),
                    in1=v.unsqueeze(1).to_broadcast((P, NSH, 2)),
                    op=AL.mult)
                nc.gpsimd.tensor_copy(out=r_t[:, :, 2], in_=ch)
                if g == 0:
                    deferred.append((i, r_t))
                else:
                    nc.tensor.matmul(pool_ps, lhsT=cls[i], rhs=r_t.rearrange("p u c -> p (u c)"),
                                     start=(i == 0), stop=(i == NT - 1), skip_group_check=True)
            if g == 0:
                # one fused compare for all tiles, slotted behind pair-0's
                # multiplies while pair-1 is still in flight
                emit_cl_all()
                for i, r_t in deferred:
                    nc.tensor.matmul(pool_ps, lhsT=cls[i], rhs=r_t.rearrange("p u c -> p (u c)"),
                                     start=(i == 0), stop=(i == NT - 1), skip_group_check=True)

        # ---- tail: means, extraction, broadcast-add ----
        pool_sb = segp.tile([P, NSH, 3], F32)
        nc.vector.tensor_copy(out=pool_sb, in_=pool_ps.rearrange("p (u c) -> p u c", c=3))
        cnt = segp.tile([P, NSH], F32)
        nc.vector.tensor_scalar(out=cnt, in0=pool_sb[:, :, 2], scalar1=1.0, scalar2=None, op0=AL.max)
        rec = segp.tile([P, NSH], F32)
        nc.vector.reciprocal(out=rec, in_=cnt)
        msrcm = segp.tile([P, NSH], F32)
        mtgtm = segp.tile([P, NSH], F32)
        nc.vector.tensor_tensor(out=msrcm, in0=pool_sb[:, :, 0], in1=rec, op=AL.mult)
        nc.vector.tensor_tensor(out=mtgtm, in0=pool_sb[:, :, 1], in1=rec, op=AL.mult)

        msrc_ps = ppool_sm.tile([P, 4], F32, tag="sm")
        nc.tensor.matmul(msrc_ps, lhsT=s1, rhs=msrcm[:, 0:4], start=True, stop=False)
        nc.tensor.matmul(msrc_ps, lhsT=s2, rhs=msrcm[:, 1:5], start=False, stop=True)
        msrc = segp.tile([P, 4], F32)
        nc.vector.tensor_scalar(out=msrc, in0=msrc_ps, scalar1=biascol, scalar2=None, op0=AL.add)

        # rowb[p, j] = mtgt mean of segment 513+j, broadcast across partitions
        # by step-0 stationary matmuls (no [1,512] row stage)
        rowb_ps = ppool_sm.tile([P, 512], F32, tag="sm")
        nc.tensor.matmul(rowb_ps[:, 0:127], lhsT=mtgtm[:, 4:5].to_broadcast((P, P)),
                         rhs=ident[:, 1:128], start=True, stop=True)
        nc.tensor.matmul(rowb_ps[:, 127:255], lhsT=mtgtm[:, 5:6].to_broadcast((P, P)),
                         rhs=ident, start=True, stop=True)
        nc.tensor.matmul(rowb_ps[:, 255:383], lhsT=mtgtm[:, 6:7].to_broadcast((P, P)),
                         rhs=ident, start=True, stop=True)
        nc.tensor.matmul(rowb_ps[:, 383:511], lhsT=mtgtm[:, 7:8].to_broadcast((P, P)),
                         rhs=ident, start=True, stop=True)
        nc.tensor.matmul(rowb_ps[:, 511:512], lhsT=mtgtm[:, 8:9].to_broadcast((P, P)),
                         rhs=ident[:, 0:1], start=True, stop=True)

        for k in range(4):
            lg = opool.tile([P, 512], F32)
            if k % 2 == 0:
                nc.scalar.activation(out=lg, in_=rowb_ps, func=mybir.ActivationFunctionType.Identity,
                                     bias=msrc[:, k:k + 1], scale=1.0)
            else:
                nc.vector.tensor_scalar(out=lg, in0=rowb_ps, scalar1=msrc[:, k:k + 1],
                                        scalar2=None, op0=AL.add)
            nc.sync.dma_start(out=y_d[P * k:P * (k + 1), :], in_=lg)

    nc.compile()
    return nc


def _host_prep(inputs):
    x = np.ascontiguousarray(np.asarray(inputs["outputs"], dtype=np.float32))
    wid = np.asarray(inputs["word_ids"]).astype(np.int64)
    cw = np.asarray(inputs["classifier_w"], dtype=np.float32)
    bias = np.float32(np.asarray(inputs["classifier_b"]))
    B, L, Hd = x.shape
    assert (Hd, L) == (H, 4096) and B == 8
    assert int(inputs["num_src"]) == 512 and int(inputs["num_tgt"]) == 512

    # token cutoff: segments beyond 1024 never reach the output
    new_seg = np.ones((B, L), np.int64)
    new_seg[:, 1:] = wid[:, 1:] != wid[:, :-1]
    seg = np.cumsum(new_seg, axis=1) - 1
    cutoff = max(int(np.nonzero(seg[b] <= 1024)[0][-1]) for b in range(B))
    NT = min((cutoff + 1 + P - 1) // P, L // P)
    NT += NT % 2  # even tile count for paired DMA
    NT = min(NT, L // P)
    Ltok = NT * P

    # per-tile projection mode (same compiled program for all cores -> union)
    modes = []
    for i in range(NT):
        smin = int(seg[:, i * P].min())
        smax = int(seg[:, i * P + P - 1].max())
        if smax <= 512:
            modes.append("src")
        elif smin >= 513:
            modes.append("tgt")
        else:
            modes.append("both")

    wrep_b = np.broadcast_to(cw, (P, 2 * H)).astype(np.float32)
    ident = np.eye(P, dtype=np.float32)
    s1 = np.eye(P, k=-1, dtype=np.float32)                      # s1[q,p]=1 iff q==p+1
    s2 = np.zeros((P, P), np.float32)
    s2[0, P - 1] = 1.0
    iota = np.broadcast_to(np.arange(P, dtype=np.float32), (P, P)).copy()

    in_maps = []
    for b in range(B):
        segt = seg[b, :Ltok].reshape(NT, P).T             # [128, NT], token 128i+p at [p, i]
        shi = np.minimum(segt // P, NSH)
        slo_t = (segt - shi * P).astype(np.float32)       # seg%128; out-of-range rows match nothing below
        ch = np.zeros((P, NT, NSH), np.float32)           # s_hi one-hot (zero for seg >= 128*NSH)
        pp, ii = np.nonzero(shi < NSH)
        ch[pp, ii, shi[pp, ii]] = 1.0
        slo_t[shi == NSH] = -1.0                          # never equal to iota 0..127
        biascol = np.full((P, 1), bias, np.float32)
        cc = np.concatenate([ident, s1, s2, iota, slo_t, ch.reshape(P, NT * NSH), biascol], axis=1)
        in_maps.append({
            "x": np.ascontiguousarray(x[b, :Ltok]),
            "consts": np.ascontiguousarray(cc),
            "wrepb": np.ascontiguousarray(wrep_b),
        })
    return NT, modes, in_maps


def _run(inputs, trace=False, tmpdir=None):
    NT, modes, in_maps = _host_prep(inputs)
    nc = _build_nc(NT, modes)
    res = run_bass_kernel_spmd(nc, in_maps, core_ids=list(range(8)), trace=trace, tmpdir=tmpdir)
    out = np.stack([np.asarray(r["y"], dtype=np.float32) for r in res.results])
    return out, res


def kernel(**inputs) -> np.ndarray:
    out, _ = _run(inputs, trace=False)
    return out


if __name__ == "__main__":
    # CoreSim smoke test on core 0's inputs
    import jax
    jax.config.update("jax_platforms", "cpu")
    sys.path.insert(0, "/root/problem")
    import reference as ref
    from concourse.bass_interp import CoreSim

    inputs = ref.setup_inputs()
    NT, modes, in_maps = _host_prep(inputs)
    print("NT =", NT, "modes:", modes)
    nc = _build_nc(NT, modes)
    sim = CoreSim(nc)
    for name, arr in in_maps[0].items():
        sim.tensor(name)[:] = arr
    sim.simulate()
    got = np.array(sim.tensor("y"))
    expected = np.asarray(ref.reference(**inputs))[0]
    err = np.abs(got - expected).max()
    scale = np.abs(expected).max()
    print("CoreSim abs err:", err, "rel:", err / scale)
    assert err / scale < 1e-2, "CoreSim mismatch"
    print("CORESIM PASSES")



# revision 6
# speedup vs baseline: 1.3154x; 1.3154x over previous
"""Trainium2 Bass kernel for nn_BinaryTokenClassificationModel (segment_reduce).

Math: logits[b,i,j] = dot(segmean(1+i), w_src) + dot(segmean(513+j), w_tgt) + bias,
where segmean(s) is the mean of outputs[b] over the s-th consecutive run of equal
word_ids (attention_mask is all ones here).  dot commutes with the segment mean,
so per-token dots v[t,c] = x[t].w_c suffice; segment sums of v are accumulated by
PE one-hot matmuls and scaled by host-computed 1/count at the very end.

Design (v1, DMA-roofline oriented):
  - Only tokens of segments 1..1024 are staged (host gathers them REVERSED, so
    tgt segments 1024..513 stream first, then src 512..1).  NT = ceil(max/128)
    tiles of 128 tokens; short examples padded with slo=-1 dummies.
  - x is cast f32->bf16 during the SWDGE (gpsimd) DMA: HBM reads stay f32 (the
    mandatory roofline) but all on-chip compute runs at 2x 16-bit rates.  All
    x-tile DMAs are issued up-front so the HBM stream is continuous.
  - Per tile: DVE multiplies x by the replicated weight row (bf16 2x), ACT's
    fused activation-accumulate reduces to per-token dots v.  PE accumulates
    pool[s_lo, u] += onehot(s_lo)^T @ (v masked per s_hi group u) into two tiny
    PSUM regions (tgt: u 4..8, src: u 0..4).  Masks (r_t) are built only for
    tiles spanning >1 u or containing both channels.
  - Counts never touch the device loop: host bakes 1/count into tiny [128,5]
    tables applied once at the tail.  The tgt half of the tail (the broadcast
    row of the output) runs early, hidden under the src-phase DMA stream.
  - Output is written bf16 (tolerance 2e-2; bf16 error ~4e-3) and upcast on host.

Sharding: pure data parallel, one example (B=8) per NeuronCore (8 cores).
"""
import sys

for _p in ("/opt/trn_rl_repo", "/root/.axon_site/_ro/trn_rl_repo"):
    if _p not in sys.path:
        sys.path.append(_p)

from contextlib import ExitStack

import ml_dtypes
import numpy as np

import concourse.bacc as bacc
import concourse.bass as bass
import concourse.tile as tile
from concourse import mybir
from concourse.bass_utils import run_bass_kernel_spmd

F32 = mybir.dt.float32
BF16 = mybir.dt.bfloat16
P = 128
H = 1024
AL = mybir.AluOpType
ACTF = mybir.ActivationFunctionType


def _build_nc(NT: int, ops: list, CW: int, lt_tgt: int, lt_src: int) -> bass.Bass:
    NCF = P + NT + CW + 16  # iota | slo | ch_all | zeros5 | rcnt_src | rcnt_tgt | bias
    nc = bacc.Bacc("TRN2", target_bir_lowering=False, debug=False, num_devices=8)
    x_d = nc.declare_dram_parameter("x", [NT * P, H], F32, isOutput=False)
    cf_d = nc.declare_dram_parameter("consts", [P, NCF], F32, isOutput=False)
    cb_d = nc.declare_dram_parameter("cbf", [P, 3 * P], BF16, isOutput=False)
    wb_d = nc.declare_dram_parameter("wrepb", [P, 2 * H], BF16, isOutput=False)
    y_d = nc.declare_dram_parameter("y", [512, 512], BF16, isOutput=True)

    with tile.TileContext(nc) as tc, ExitStack() as ctx:
        consts = ctx.enter_context(tc.tile_pool(name="consts", bufs=1))
        clp = ctx.enter_context(tc.tile_pool(name="clp", bufs=1))
        xpool = ctx.enter_context(tc.tile_pool(name="xp", bufs=1))
        scrp = ctx.enter_context(tc.tile_pool(name="scr", bufs=3))
        vpool = ctx.enter_context(tc.tile_pool(name="vp", bufs=8))
        rpool = ctx.enter_context(tc.tile_pool(name="rp", bufs=4))
        segp = ctx.enter_context(tc.tile_pool(name="segp", bufs=1))
        opool = ctx.enter_context(tc.tile_pool(name="op", bufs=4))
        pp_pool = ctx.enter_context(tc.tile_pool(name="ppool", bufs=1, space="PSUM"))
        pp_row = ctx.enter_context(tc.tile_pool(name="prow", bufs=1, space="PSUM"))
        pp_ms = ctx.enter_context(tc.tile_pool(name="pms", bufs=1, space="PSUM"))

        # ---- HWDGE (sync) queue: weights + consts, later the output ----
        wrep = consts.tile([P, 2 * H], BF16)
        nc.sync.dma_start(out=wrep[:, H:2 * H], in_=wb_d[:, H:2 * H])  # w_tgt first
        cf = consts.tile([P, NCF], F32)
        nc.sync.dma_start(out=cf, in_=cf_d[:])
        cb = consts.tile([P, 3 * P], BF16)
        nc.sync.dma_start(out=cb, in_=cb_d[:])
        nc.sync.dma_start(out=wrep[:, 0:H], in_=wb_d[:, 0:H])

        iota = cf[:, 0:P]
        slo = cf[:, P:P + NT]
        ch_all = cf[:, P + NT:P + NT + CW]
        zeros5 = cf[:, NCF - 16:NCF - 11]
        rcS = cf[:, NCF - 11:NCF - 6]
        rcT = cf[:, NCF - 6:NCF - 1]
        biascol = cf[:, NCF - 1:NCF]
        s1 = cb[:, 0:P]
        s2 = cb[:, P:2 * P]
        ident = cb[:, 2 * P:3 * P]

        # ---- PSUM pools, zero-initialized via start=True matmuls ----
        pool_t = pp_pool.tile([P, 5], F32)  # tgt sums: col j = u-4, row = s%128
        pool_s = pp_pool.tile([P, 5], F32)  # src sums: col j = u,   row = s%128
        nc.tensor.matmul(pool_t, lhsT=iota, rhs=zeros5, start=True, stop=False,
                         skip_group_check=True)
        nc.tensor.matmul(pool_s, lhsT=iota, rhs=zeros5, start=True, stop=False,
                         skip_group_check=True)

        # ---- x stream: all tiles up-front on the SWDGE (gpsimd) queue, cast
        # f32->bf16 in the DMA datapath.  cls build slots in after 3 tiles. ----
        x_tiles = [xpool.tile([P, H], BF16, name=f"xt{i}") for i in range(NT)]
        cl_all = clp.tile([P, NT, P], F32)
        for i in range(NT):
            nc.gpsimd.dma_start(out=x_tiles[i], in_=x_d[P * i:P * (i + 1), :])
        nc.vector.tensor_tensor(
            out=cl_all,
            in0=iota.unsqueeze(1).to_broadcast((P, NT, P)),
            in1=slo.unsqueeze(2).to_broadcast((P, NT, P)),
            op=AL.is_equal)

        # ---- main loop over token tiles ----
        rowb_sb = segp.tile([P, 512], BF16)
        for i in range(NT):
            for e in ops[i]:
                c01 = 1 if e["c"] == "tgt" else 0
                scr = scrp.tile([P, H], BF16)
                nc.vector.tensor_tensor(out=scr, in0=x_tiles[i],
                                        in1=wrep[:, c01 * H:(c01 + 1) * H], op=AL.mult)
                v = vpool.tile([P, 1], F32)
                nc.scalar.activation(out=scr, in_=scr, func=ACTF.Copy, accum_out=v)
                nU = len(e["ulist"])
                if e["direct"]:
                    rhs = v
                else:
                    r_t = rpool.tile([P, nU], F32)
                    off = e["ch_off"]
                    nc.vector.tensor_tensor(out=r_t, in0=ch_all[:, off:off + nU],
                                            in1=v.to_broadcast((P, nU)), op=AL.mult)
                    rhs = r_t
                if e["c"] == "tgt":
                    pool, col_lo, stop = pool_t, e["ulist"][0] - 4, i == lt_tgt
                else:
                    pool, col_lo, stop = pool_s, e["ulist"][0], i == lt_src
                nc.tensor.matmul(pool[:, col_lo:col_lo + nU], lhsT=cl_all[:, i, :],
                                 rhs=rhs, start=False, stop=stop, skip_group_check=True)
            if i == lt_tgt:
                # tgt tail early: broadcast row of the output, hidden under the
                # src-phase DMA stream
                mtgtm = segp.tile([P, 5], BF16)
                nc.vector.tensor_tensor(out=mtgtm, in0=pool_t, in1=rcT, op=AL.mult)
                rowb_ps = pp_row.tile([P, 512], F32)
                nc.tensor.matmul(rowb_ps[:, 0:127], lhsT=mtgtm[:, 0:1].to_broadcast((P, P)),
                                 rhs=ident[:, 1:128], start=True, stop=True)
                nc.tensor.matmul(rowb_ps[:, 127:255], lhsT=mtgtm[:, 1:2].to_broadcast((P, P)),
                                 rhs=ident, start=True, stop=True)
                nc.tensor.matmul(rowb_ps[:, 255:383], lhsT=mtgtm[:, 2:3].to_broadcast((P, P)),
                                 rhs=ident, start=True, stop=True)
                nc.tensor.matmul(rowb_ps[:, 383:511], lhsT=mtgtm[:, 3:4].to_broadcast((P, P)),
                                 rhs=ident, start=True, stop=True)
                nc.tensor.matmul(rowb_ps[:, 511:512], lhsT=mtgtm[:, 4:5].to_broadcast((P, P)),
                                 rhs=ident[:, 0:1], start=True, stop=True)
                nc.scalar.activation(out=rowb_sb, in_=rowb_ps, func=ACTF.Identity,
                                     bias=biascol, scale=1.0)

        # ---- src tail: shift matmuls + broadcast-add + store ----
        msrcm = segp.tile([P, 5], BF16)
        nc.vector.tensor_tensor(out=msrcm, in0=pool_s, in1=rcS, op=AL.mult)
        msrc_ps = pp_ms.tile([P, 4], F32)
        nc.tensor.matmul(msrc_ps, lhsT=s1, rhs=msrcm[:, 0:4], start=True, stop=False)
        nc.tensor.matmul(msrc_ps, lhsT=s2, rhs=msrcm[:, 1:5], start=False, stop=True)
        msrc_sb = segp.tile([P, 4], F32)
        nc.vector.tensor_copy(out=msrc_sb, in_=msrc_ps)
        for k in range(4):
            lg = opool.tile([P, 512], BF16)
            nc.vector.tensor_scalar(out=lg, in0=rowb_sb, scalar1=msrc_sb[:, k:k + 1],
                                    scalar2=None, op0=AL.add)
            nc.sync.dma_start(out=y_d[P * k:P * (k + 1), :], in_=lg)

    nc.compile()
    return nc


def _host_prep(inputs):
    x = np.asarray(inputs["outputs"], dtype=np.float32)
    wid = np.asarray(inputs["word_ids"]).astype(np.int64)
    cw = np.asarray(inputs["classifier_w"], dtype=np.float32)
    bias = np.float32(np.asarray(inputs["classifier_b"]))
    B, L, Hd = x.shape
    assert (Hd, L, B) == (H, 4096, 8)
    assert int(inputs["num_src"]) == 512 and int(inputs["num_tgt"]) == 512
    assert np.asarray(inputs["attention_mask"]).min() == 1

    segs, idxs = [], []
    for b in range(B):
        ns = np.ones(L, np.int64)
        ns[1:] = wid[b, 1:] != wid[b, :-1]
        seg = np.cumsum(ns) - 1
        keep = (seg >= 1) & (seg <= 1024)
        idxs.append(np.nonzero(keep)[0][::-1])  # descending segment order
        segs.append(seg)
    ntoks = [len(i) for i in idxs]
    NT = (max(ntoks) + P - 1) // P
    L2 = NT * P

    tok_s = np.full((B, L2), -1, np.int64)
    xbs = []
    for b in range(B):
        n = ntoks[b]
        tok_s[b, :n] = segs[b][idxs[b]]
        xi = np.zeros(L2, np.int64)
        xi[:n] = idxs[b]
        xbs.append(np.ascontiguousarray(x[b][xi]))

    is_t = tok_s >= 513
    is_s = (tok_s >= 1) & (tok_s <= 512)
    u = np.where(tok_s >= 0, tok_s >> 7, -1)
    slo_v = np.where(tok_s >= 0, tok_s & 127, -1)

    # program metadata, unioned over cores (same compiled program everywhere)
    ops, CW, ch_cols = [], 0, []
    for i in range(NT):
        sl = slice(i * P, (i + 1) * P)
        ent = []
        for cname, m in (("tgt", is_t), ("src", is_s)):
            msk = m[:, sl]
            if not msk.any():
                continue
            uu = u[:, sl][msk]
            ulist = list(range(int(uu.min()), int(uu.max()) + 1))
            assert len(ulist) <= 3
            other = (is_s if cname == "tgt" else is_t)[:, sl].any()
            direct = (len(ulist) == 1) and not other
            d = dict(c=cname, ulist=ulist, direct=direct, ch_off=None)
            if not direct:
                d["ch_off"] = CW
                for uv in ulist:
                    ch_cols.append((i, cname, uv))
                CW += len(ulist)
            ent.append(d)
        ops.append(ent)
    lt_tgt = max(i for i in range(NT) if any(e["c"] == "tgt" for e in ops[i]))
    lt_src = max(i for i in range(NT) if any(e["c"] == "src" for e in ops[i]))

    iota_h = np.broadcast_to(np.arange(P, dtype=np.float32), (P, P))
    s1_h = np.eye(P, k=-1, dtype=np.float32)  # s1[p,m]=1 iff m==p-1 -> out[m]=in[m+1]
    s2_h = np.zeros((P, P), np.float32)
    s2_h[0, P - 1] = 1.0
    ident_h = np.eye(P, dtype=np.float32)
    cb_h = np.concatenate([s1_h, s2_h, ident_h], axis=1).astype(ml_dtypes.bfloat16)
    wrep_h = np.broadcast_to(cw, (P, 2 * H)).astype(ml_dtypes.bfloat16)

    in_maps = []
    for b in range(B):
        cnt = np.bincount(tok_s[b][tok_s[b] >= 0], minlength=1025).astype(np.float64)
        rcS_h = np.ones((P, 5), np.float32)
        rcT_h = np.ones((P, 5), np.float32)
        for j in range(5):
            for p in range(P):
                s_src = 128 * j + p
                if 1 <= s_src <= 512:
                    rcS_h[p, j] = 1.0 / max(cnt[s_src], 1.0)
                s_tgt = 128 * (j + 4) + p
                if 513 <= s_tgt <= 1024:
                    rcT_h[p, j] = 1.0 / max(cnt[s_tgt], 1.0)
        slo_t = slo_v[b].reshape(NT, P).T.astype(np.float32)  # [128, NT]
        ch_h = np.zeros((P, CW), np.float32)
        for k, (i, cname, uv) in enumerate(ch_cols):
            m = (is_t if cname == "tgt" else is_s)[b, i * P:(i + 1) * P]
            ch_h[:, k] = (m & (u[b, i * P:(i + 1) * P] == uv)).astype(np.float32)
        zeros5 = np.zeros((P, 5), np.float32)
        biascol = np.full((P, 1), bias, np.float32)
        cf_h = np.concatenate(
            [iota_h, slo_t, ch_h, zeros5, rcS_h, rcT_h, biascol], axis=1)
        in_maps.append({
            "x": xbs[b],
            "consts": np.ascontiguousarray(cf_h.astype(np.float32)),
            "cbf": np.ascontiguousarray(cb_h),
            "wrepb": np.ascontiguousarray(wrep_h),
        })
    return NT, ops, CW, lt_tgt, lt_src, in_maps


def _run(inputs, trace=False, tmpdir=None):
    NT, ops, CW, lt_tgt, lt_src, in_maps = _host_prep(inputs)
    nc = _build_nc(NT, ops, CW, lt_tgt, lt_src)
    res = run_bass_kernel_spmd(nc, in_maps, core_ids=list(range(8)), trace=trace, tmpdir=tmpdir)
    out = np.stack([np.asarray(r["y"]).astype(np.float32) for r in res.results])
    return out, res


def kernel(**inputs) -> np.ndarray:
    out, _ = _run(inputs, trace=False)
    return out


if __name__ == "__main__":
    # CoreSim smoke test on core 0's inputs
    import jax
    jax.config.update("jax_platforms", "cpu")
    sys.path.insert(0, "/root/problem")
    import reference as ref
    from concourse.bass_interp import CoreSim

    inputs = ref.setup_inputs()
    NT, ops, CW, lt_tgt, lt_src, in_maps = _host_prep(inputs)
    print("NT =", NT, "CW =", CW, "lt_tgt =", lt_tgt, "lt_src =", lt_src)
    for i, ent in enumerate(ops):
        print(i, [(e["c"], e["ulist"], e["direct"]) for e in ent])
    nc = _build_nc(NT, ops, CW, lt_tgt, lt_src)
    sim = CoreSim(nc)
    for name, arr in in_maps[0].items():
        sim.tensor(name)[:] = arr
    sim.simulate()
    got = np.array(sim.tensor("y")).astype(np.float32)
    expected = np.asarray(ref.reference(**inputs))[0]
    err = np.abs(got - expected).max()
    scale = np.abs(expected).max()
    print("CoreSim abs err:", err, "rel:", err / scale)
    assert err / scale < 1e-2, "CoreSim mismatch"
    print("CORESIM PASSES")


# revision 11
# speedup vs baseline: 1.3559x; 1.0308x over previous
"""Trainium2 Bass kernel for nn_BinaryTokenClassificationModel (segment_reduce).

Math: logits[b,i,j] = dot(segmean(1+i), w_src) + dot(segmean(513+j), w_tgt) + bias,
where segmean(s) is the mean of outputs[b] over the s-th consecutive run of equal
word_ids (attention_mask is all ones here).  dot commutes with the segment mean,
so per-token dots v[t,c] = x[t].w_c suffice; segment sums of v are accumulated by
PE one-hot matmuls and scaled by host-computed 1/count at the very end.

Design (v3, DMA-roofline oriented):
  - Only tokens of segments 1..1024 are staged (host gathers them REVERSED, so
    tgt segments 1024..513 stream first, then src 512..1).  NT = ceil(max/128)
    tiles of 128 tokens; short examples padded with slo=-1 dummies.
  - x is cast f32->bf16 during the SWDGE (gpsimd) DMA: HBM reads stay f32 (the
    mandatory roofline) but on-chip compute runs at 16-bit rates.  All x DMAs
    are issued up-front in 8 chunks so the HBM stream is continuous.
  - Per tile: DVE tensor_tensor multiplies x by the replicated weight row (bf16
    2x mode, ~690ns); the h-reduction to v is split between ACT's fused
    activation-accumulate and DVE tensor_reduce so neither engine exceeds the
    DMA stream time.  gpsimd builds the tiny per-u mask r_t = ch*v (bf16), and
    PE accumulates pool[s_lo, u] += onehot(s_lo)^T @ r_t with all-bf16 matmuls
    into small PSUM regions (tgt: u 4..8, src: u 0..4, late-src: u 0).
  - Counts never touch the device loop: host bakes 1/count into tiny [128,5]
    tables applied at the tail.  The tgt half of the output (broadcast row) and
    blocks 1-3 are emitted early, hidden under the src-phase DMA stream; only
    block 0 (which needs the last tiles) remains in the tail.
  - Output is written bf16 (tolerance 2e-2; bf16 error ~5e-3) and upcast on host.

Sharding: pure data parallel, one example (B=8) per NeuronCore (8 cores).
"""
import sys

for _p in ("/opt/trn_rl_repo", "/root/.axon_site/_ro/trn_rl_repo"):
    if _p not in sys.path:
        sys.path.append(_p)

from contextlib import ExitStack

import ml_dtypes
import numpy as np

import concourse.bacc as bacc
import concourse.bass as bass
import concourse.tile as tile
from concourse import mybir
from concourse.bass_utils import run_bass_kernel_spmd

F32 = mybir.dt.float32
BF16 = mybir.dt.bfloat16
P = 128
H = 1024
AL = mybir.AluOpType
ACTF = mybir.ActivationFunctionType

# x-tile DMA chunking (tiles per SWDGE dma_start); last chunks small to keep
# the post-stream tail short
def _chunks_for(NT):
    sizes = []
    rem = NT
    plan = [2, 3, 3, 3, 3, 3]
    for s in plan:
        if rem <= 2:
            break
        k = min(s, rem - 1)
        sizes.append(k)
        rem -= k
    sizes += [1] * rem
    starts = np.cumsum([0] + sizes[:-1]).tolist()
    return list(zip(starts, sizes))


def _build_nc(NT: int, ops: list, CW: int, lt_tgt: int, lt_s1: int, lt_src: int) -> bass.Bass:
    NCF = CW + 11          # ch_all | rcnt_src | rcnt_tgt | bias
    NCB = 4 * P + NT + 8   # s1 | s2 | ident | iota | slo | zeros(8)
    nc = bacc.Bacc("TRN2", target_bir_lowering=False, debug=False, num_devices=8)
    x_d = nc.declare_dram_parameter("x", [NT * P, H], F32, isOutput=False)
    cf_d = nc.declare_dram_parameter("consts", [P, NCF], F32, isOutput=False)
    cb_d = nc.declare_dram_parameter("cbf", [P, NCB], BF16, isOutput=False)
    wb_d = nc.declare_dram_parameter("wrepb", [P, 2 * H], BF16, isOutput=False)
    y_d = nc.declare_dram_parameter("y", [512, 512], BF16, isOutput=True)

    with tile.TileContext(nc) as tc, ExitStack() as ctx:
        consts = ctx.enter_context(tc.tile_pool(name="consts", bufs=1))
        clp = ctx.enter_context(tc.tile_pool(name="clp", bufs=1))
        xpool = ctx.enter_context(tc.tile_pool(name="xp", bufs=1))
        scrp = ctx.enter_context(tc.tile_pool(name="scr", bufs=3))
        vpool = ctx.enter_context(tc.tile_pool(name="vp", bufs=8))
        rpool = ctx.enter_context(tc.tile_pool(name="rp", bufs=4))
        segp = ctx.enter_context(tc.tile_pool(name="segp", bufs=1))
        opool = ctx.enter_context(tc.tile_pool(name="op", bufs=4))
        pp_pool = ctx.enter_context(tc.tile_pool(name="ppool", bufs=1, space="PSUM"))
        pp_row = ctx.enter_context(tc.tile_pool(name="prow", bufs=1, space="PSUM"))
        pp_ms = ctx.enter_context(tc.tile_pool(name="pms", bufs=1, space="PSUM"))

        # ---- HWDGE (sync) queue: weights + consts, later the output ----
        wrep = consts.tile([P, 2 * H], BF16)
        nc.sync.dma_start(out=wrep[:, H:2 * H], in_=wb_d[:, H:2 * H])  # w_tgt first
        cb = consts.tile([P, NCB], BF16)
        nc.sync.dma_start(out=cb, in_=cb_d[:])
        cf = consts.tile([P, NCF], F32)
        nc.sync.dma_start(out=cf, in_=cf_d[:])
        nc.sync.dma_start(out=wrep[:, 0:H], in_=wb_d[:, 0:H])

        ch_all = cf[:, 0:CW]
        rcS = cf[:, CW:CW + 5]
        rcT = cf[:, CW + 5:CW + 10]
        biascol = cf[:, CW + 10:CW + 11]
        s1 = cb[:, 0:P]
        s2 = cb[:, P:2 * P]
        ident = cb[:, 2 * P:3 * P]
        iota = cb[:, 3 * P:4 * P]
        slo = cb[:, 4 * P:4 * P + NT]
        zeros8 = cb[:, 4 * P + NT:4 * P + NT + 8]

        # ---- PSUM pools, zero-initialized via start=True matmuls ----
        # pool_s2 takes the src contributions of tiles after lt_s1 (they only
        # touch u=0), so pool_s closes early and blocks 1-3 can be emitted
        # while the x stream is still running.
        pool_t = pp_pool.tile([P, 5], F32)  # tgt sums: col j = u-4, row = s%128
        pool_s = pp_pool.tile([P, 5], F32)  # src sums: col j = u,   row = s%128
        pool_s2 = pp_pool.tile([P, 1], F32)
        nc.tensor.matmul(pool_t, lhsT=iota, rhs=zeros8[:, 0:5], start=True,
                         stop=False, skip_group_check=True)
        nc.tensor.matmul(pool_s, lhsT=iota, rhs=zeros8[:, 0:5], start=True,
                         stop=False, skip_group_check=True)
        nc.tensor.matmul(pool_s2, lhsT=iota, rhs=zeros8[:, 0:1], start=True,
                         stop=False, skip_group_check=True)

        # ---- x stream: all chunks up-front on the SWDGE (gpsimd) queue,
        # cast f32->bf16 in the DMA datapath ----
        chunks = _chunks_for(NT)
        x_tiles = [None] * NT
        for c, (st, k) in enumerate(chunks):
            xc = xpool.tile([P, k, H], BF16, name=f"xc{c}")
            nc.gpsimd.dma_start(
                out=xc, in_=x_d[P * st:P * (st + k), :].rearrange("(k p) h -> p k h", p=P))
            for j in range(k):
                x_tiles[st + j] = xc[:, j, :]

        cl_all = clp.tile([P, NT, P], BF16)
        nc.vector.tensor_tensor(
            out=cl_all,
            in0=iota.unsqueeze(1).to_broadcast((P, NT, P)),
            in1=slo.unsqueeze(2).to_broadcast((P, NT, P)),
            op=AL.is_equal)

        # ---- main loop over token tiles ----
        rowb_sb = segp.tile([P, 512], BF16)
        msrcm14 = segp.tile([P, 5], BF16)

        def emit_block(k, rhs1, rhs2):
            msps = pp_ms.tile([P, 1], F32, name=f"msps{k}")
            nc.tensor.matmul(msps, lhsT=s1, rhs=rhs1, start=True, stop=False,
                             skip_group_check=True)
            nc.tensor.matmul(msps, lhsT=s2, rhs=rhs2, start=False, stop=True,
                             skip_group_check=True)
            msv = segp.tile([P, 1], F32, name=f"msv{k}")
            nc.vector.tensor_copy(out=msv, in_=msps)
            lg = opool.tile([P, 512], BF16, name=f"lg{k}")
            nc.vector.tensor_scalar(out=lg, in0=rowb_sb, scalar1=msv,
                                    scalar2=None, op0=AL.add)
            nc.sync.dma_start(out=y_d[P * k:P * (k + 1), :], in_=lg)

        for i in range(NT):
            for e in ops[i]:
                c01 = 1 if e["c"] == "tgt" else 0
                scr = scrp.tile([P, H], BF16)
                v = vpool.tile([P, 1], F32)
                nc.vector.tensor_tensor(out=scr, in0=x_tiles[i],
                                        in1=wrep[:, c01 * H:(c01 + 1) * H], op=AL.mult)
                if e["red"] == "dve":
                    nc.vector.tensor_reduce(out=v, in_=scr, axis=mybir.AxisListType.X,
                                            op=AL.add)
                else:
                    nc.scalar.activation(out=scr, in_=scr, func=ACTF.Copy, accum_out=v)
                nU = len(e["ulist"])
                r_t = rpool.tile([P, nU], BF16)
                off = e["ch_off"]
                nc.gpsimd.tensor_tensor(out=r_t, in0=ch_all[:, off:off + nU],
                                        in1=v.to_broadcast((P, nU)), op=AL.mult)
                if e["c"] == "tgt":
                    pool, col_lo, stop = pool_t, e["ulist"][0] - 4, i == lt_tgt
                elif i <= lt_s1:
                    pool, col_lo, stop = pool_s, e["ulist"][0], i == lt_s1
                else:
                    assert e["ulist"] == [0]
                    pool, col_lo, stop = pool_s2, 0, i == lt_src
                nc.tensor.matmul(pool[:, col_lo:col_lo + nU], lhsT=cl_all[:, i, :],
                                 rhs=r_t, start=False, stop=stop, skip_group_check=True)
            if i == lt_tgt:
                # tgt tail early: broadcast row of the output, hidden under the
                # src-phase DMA stream
                mtgtm = segp.tile([P, 5], BF16)
                nc.vector.tensor_tensor(out=mtgtm, in0=pool_t, in1=rcT, op=AL.mult)
                rowb_ps = pp_row.tile([P, 512], F32)
                nc.tensor.matmul(rowb_ps[:, 0:127], lhsT=mtgtm[:, 0:1].to_broadcast((P, P)),
                                 rhs=ident[:, 1:128], start=True, stop=True)
                nc.tensor.matmul(rowb_ps[:, 127:255], lhsT=mtgtm[:, 1:2].to_broadcast((P, P)),
                                 rhs=ident, start=True, stop=True)
                nc.tensor.matmul(rowb_ps[:, 255:383], lhsT=mtgtm[:, 2:3].to_broadcast((P, P)),
                                 rhs=ident, start=True, stop=True)
                nc.tensor.matmul(rowb_ps[:, 383:511], lhsT=mtgtm[:, 3:4].to_broadcast((P, P)),
                                 rhs=ident, start=True, stop=True)
                nc.tensor.matmul(rowb_ps[:, 511:512], lhsT=mtgtm[:, 4:5].to_broadcast((P, P)),
                                 rhs=ident[:, 0:1], start=True, stop=True)
                nc.scalar.activation(out=rowb_sb, in_=rowb_ps, func=ACTF.Identity,
                                     bias=biascol, scale=1.0)
            if i == lt_s1:
                # pool_s closed: blocks 1-3 emitted under the x stream
                nc.vector.tensor_tensor(out=msrcm14, in0=pool_s, in1=rcS, op=AL.mult)
                for k in (1, 2, 3):
                    emit_block(k, msrcm14[:, k:k + 1], msrcm14[:, k + 1:k + 2])

        # ---- final tail: only block 0 (needs the trailing u=0 tiles) ----
        msrcm0 = segp.tile([P, 1], BF16)
        if lt_src > lt_s1:
            nc.vector.tensor_scalar(out=msrcm0, in0=pool_s[:, 0:1], scalar1=pool_s2,
                                    scalar2=rcS[:, 0:1], op0=AL.add, op1=AL.mult)
        else:
            nc.vector.tensor_tensor(out=msrcm0, in0=pool_s[:, 0:1], in1=rcS[:, 0:1],
                                    op=AL.mult)
        emit_block(0, msrcm0, msrcm14[:, 1:2])

    nc.compile()
    return nc


def _host_prep(inputs):
    x = np.asarray(inputs["outputs"], dtype=np.float32)
    wid = np.asarray(inputs["word_ids"]).astype(np.int64)
    cw = np.asarray(inputs["classifier_w"], dtype=np.float32)
    bias = np.float32(np.asarray(inputs["classifier_b"]))
    B, L, Hd = x.shape
    assert (Hd, L, B) == (H, 4096, 8)
    assert int(inputs["num_src"]) == 512 and int(inputs["num_tgt"]) == 512
    assert np.asarray(inputs["attention_mask"]).min() == 1

    segs, idxs = [], []
    for b in range(B):
        ns = np.ones(L, np.int64)
        ns[1:] = wid[b, 1:] != wid[b, :-1]
        seg = np.cumsum(ns) - 1
        keep = (seg >= 1) & (seg <= 1024)
        idxs.append(np.nonzero(keep)[0][::-1])  # descending segment order
        segs.append(seg)
    ntoks = [len(i) for i in idxs]
    NT = (max(ntoks) + P - 1) // P
    L2 = NT * P

    tok_s = np.full((B, L2), -1, np.int64)
    xbs = []
    for b in range(B):
        n = ntoks[b]
        tok_s[b, :n] = segs[b][idxs[b]]
        xi = np.zeros(L2, np.int64)
        xi[:n] = idxs[b]
        xbs.append(np.ascontiguousarray(x[b][xi]))

    is_t = tok_s >= 513
    is_s = (tok_s >= 1) & (tok_s <= 512)
    u = np.where(tok_s >= 0, tok_s >> 7, -1)
    slo_v = np.where(tok_s >= 0, tok_s & 127, -1)

    # program metadata, unioned over cores (same compiled program everywhere)
    ops, CW, ch_cols = [], 0, []
    for i in range(NT):
        sl = slice(i * P, (i + 1) * P)
        ent = []
        for cname, m in (("tgt", is_t), ("src", is_s)):
            msk = m[:, sl]
            if not msk.any():
                continue
            uu = u[:, sl][msk]
            ulist = list(range(int(uu.min()), int(uu.max()) + 1))
            assert len(ulist) <= 3
            d = dict(c=cname, ulist=ulist, ch_off=CW, red="act")
            for uv in ulist:
                ch_cols.append((i, cname, uv))
            CW += len(ulist)
            ent.append(d)
        ops.append(ent)
    lt_tgt = max(i for i in range(NT) if any(e["c"] == "tgt" for e in ops[i]))
    lt_src = max(i for i in range(NT) if any(e["c"] == "src" for e in ops[i]))
    lt_s1 = max(i for i in range(NT)
                if any(e["c"] == "src" and max(e["ulist"]) >= 1 for e in ops[i]))
    for i in range(lt_s1 + 1, NT):
        assert all(e["c"] == "src" and e["ulist"] == [0] for e in ops[i])
    # offload some reductions from ACT to DVE so neither engine exceeds the
    # DMA stream time
    flat = [e for ent in ops for e in ent]
    n_dve = max(0, (len(flat) * 2) // 8)  # ~1/4 of reduces on DVE
    for j in np.linspace(2, len(flat) - 2, n_dve).astype(int):
        flat[int(j)]["red"] = "dve"

    iota_h = np.broadcast_to(np.arange(P, dtype=np.float32), (P, P))
    s1_h = np.eye(P, k=-1, dtype=np.float32)  # s1[p,m]=1 iff m==p-1 -> out[m]=in[m+1]
    s2_h = np.zeros((P, P), np.float32)
    s2_h[0, P - 1] = 1.0
    ident_h = np.eye(P, dtype=np.float32)
    wrep_h = np.broadcast_to(cw, (P, 2 * H)).astype(ml_dtypes.bfloat16)

    in_maps = []
    for b in range(B):
        cnt = np.bincount(tok_s[b][tok_s[b] >= 0], minlength=1025).astype(np.float64)
        rcS_h = np.ones((P, 5), np.float32)
        rcT_h = np.ones((P, 5), np.float32)
        for j in range(5):
            for p in range(P):
                s_src = 128 * j + p
                if 1 <= s_src <= 512:
                    rcS_h[p, j] = 1.0 / max(cnt[s_src], 1.0)
                s_tgt = 128 * (j + 4) + p
                if 513 <= s_tgt <= 1024:
                    rcT_h[p, j] = 1.0 / max(cnt[s_tgt], 1.0)
        slo_t = slo_v[b].reshape(NT, P).T.astype(np.float32)  # [128, NT]
        ch_h = np.zeros((P, CW), np.float32)
        for k, (i, cname, uv) in enumerate(ch_cols):
            m = (is_t if cname == "tgt" else is_s)[b, i * P:(i + 1) * P]
            ch_h[:, k] = (m & (u[b, i * P:(i + 1) * P] == uv)).astype(np.float32)
        biascol = np.full((P, 1), bias, np.float32)
        cf_h = np.concatenate([ch_h, rcS_h, rcT_h, biascol], axis=1)
        cb_h = np.concatenate(
            [s1_h, s2_h, ident_h, iota_h, slo_t, np.zeros((P, 8), np.float32)],
            axis=1).astype(ml_dtypes.bfloat16)
        in_maps.append({
            "x": xbs[b],
            "consts": np.ascontiguousarray(cf_h.astype(np.float32)),
            "cbf": np.ascontiguousarray(cb_h),
            "wrepb": np.ascontiguousarray(wrep_h),
        })
    return NT, ops, CW, lt_tgt, lt_s1, lt_src, in_maps


def _run(inputs, trace=False, tmpdir=None):
    NT, ops, CW, lt_tgt, lt_s1, lt_src, in_maps = _host_prep(inputs)
    nc = _build_nc(NT, ops, CW, lt_tgt, lt_s1, lt_src)
    res = run_bass_kernel_spmd(nc, in_maps, core_ids=list(range(8)), trace=trace, tmpdir=tmpdir)
    out = np.stack([np.asarray(r["y"]).astype(np.float32) for r in res.results])
    return out, res


def kernel(**inputs) -> np.ndarray:
    out, _ = _run(inputs, trace=False)
    return out


if __name__ == "__main__":
    # CoreSim smoke test on core 0's inputs
    import jax
    jax.config.update("jax_platforms", "cpu")
    sys.path.insert(0, "/root/problem")
    import reference as ref
    from concourse.bass_interp import CoreSim

    inputs = ref.setup_inputs()
    NT, ops, CW, lt_tgt, lt_s1, lt_src, in_maps = _host_prep(inputs)
    print("NT =", NT, "CW =", CW, "lt_tgt =", lt_tgt, "lt_s1 =", lt_s1, "lt_src =", lt_src)
    for i, ent in enumerate(ops):
        print(i, [(e["c"], e["ulist"], e["red"]) for e in ent])
    nc = _build_nc(NT, ops, CW, lt_tgt, lt_s1, lt_src)
    sim = CoreSim(nc)
    for name, arr in in_maps[0].items():
        sim.tensor(name)[:] = arr
    sim.simulate()
    got = np.array(sim.tensor("y")).astype(np.float32)
    expected = np.asarray(ref.reference(**inputs))[0]
    err = np.abs(got - expected).max()
    scale = np.abs(expected).max()
    print("CoreSim abs err:", err, "rel:", err / scale)
    assert err / scale < 1e-2, "CoreSim mismatch"
    print("CORESIM PASSES")
